# revision 39
# baseline (speedup 1.0000x reference)
"""Trainium2 Bass kernel for nn_Attention_90125593739547.

Full-input contract: kernel(**inputs) takes the unsharded numpy inputs and
returns the full [S, B, D] output. Internally:
  - 8 NeuronCores, core c handles batch b = c // 4 and 4 heads (c % 4).
  - All matmul operands are bf16 (fp32 PSUM accumulation); biases are folded
    into PSUM-evacuation vector ops; bp is added on host.
  - DRAM layouts are chunk-major so every DMA moves 4-8KB per partition row
    (descriptor-efficient).
  - Per-core program:
      phase A (per 512-seq chunk): kT = Wk.T@x (+bk), V~ = x@[Wv|0] (+[bv|1]),
      and for chunk 0 also qT; the first attention stripe (qb0, m0) is
      interleaved so the Activation engine starts exp early.
      phase B: remaining 7 stripes; qT for chunks 1-3 and the per-q-block
      projection are interleaved into the stripe stream (proj for block j is
      emitted after stripe (j+1, m0) so it never stalls the PE queue behind
      the DVE normalize chain).
      stripe (q-block 512, head-pair m):
        sc = kT_h.T-slice @ qT_h-slice per half   [128t, 2*512q] PSUM
        pT = exp(SCALE * sc)                       (ScalarE, bf16 out)
        pv[65, 1024] += V~_h.T @ pT_h              (row 64 = softmax denom)
        OT = pv[0:64] * bcast(recip(denom))        (DVE + gpsimd)
      proj: y = OT.T @ Wp (host adds bp + reduces 4 head-group partials).
"""
import sys
sys.path.insert(0, '/opt/trn_rl_repo')
import numpy as np
import ml_dtypes
from contextlib import ExitStack

S, B, D = 2048, 2, 1024
H, HD = 16, 64
SCALE = 1.0 / (HD ** 0.5)
P = 128
N_CORES = 8
CORES_PER_B = 4
NH = H // CORES_PER_B          # heads per core = 4
HDL = NH * HD                  # local head width = 256
NV = NH * 65                   # V~ width incl. denominator columns = 260
QB = 512                       # query block
CH = 512                       # seq chunk for QKV production

_cache = {}


def _build():
    import concourse.bacc as bacc
    import concourse.mybir as mybir
    from concourse import tile

    F32 = mybir.dt.float32
    BF16 = mybir.dt.bfloat16
    AF = mybir.ActivationFunctionType
    MUL = mybir.AluOpType.mult
    ADD = mybir.AluOpType.add

    n_d, n_t, n_m = D // P, S // P, HDL // P     # 8, 16, 2
    n_qb = S // QB                               # 4
    n_ch = S // CH                               # 4
    tpc = CH // P                                # seq tiles per chunk = 4
    XW = n_d * CH                                # x chunk width = 4096

    nc = bacc.Bacc("TRN2", target_bir_lowering=False, debug=False,
                   num_devices=N_CORES)

    # Chunk-major layouts (see make_in_maps for the host-side reshapes).
    x = nc.dram_tensor("x", [n_ch * P, XW], BF16, kind="ExternalInput")
    wq = nc.dram_tensor("wq", [P, n_d * HDL], BF16, kind="ExternalInput")
    wk = nc.dram_tensor("wk", [P, n_d * HDL], BF16, kind="ExternalInput")
    wv = nc.dram_tensor("wv", [P, n_d * NV], BF16, kind="ExternalInput")
    bq = nc.dram_tensor("bq", [P, n_m], F32, kind="ExternalInput")
    bk = nc.dram_tensor("bk", [P, n_m], F32, kind="ExternalInput")
    bv = nc.dram_tensor("bv", [1, NV], F32, kind="ExternalInput")
    wp = nc.dram_tensor("wp", [P, n_m * D], BF16, kind="ExternalInput")
    y = nc.dram_tensor("y", [S, D], F32, kind="ExternalOutput")
    import os as _os

    with tile.TileContext(nc) as tc, ExitStack() as ctx:
        sb = ctx.enter_context(tc.tile_pool(name="sb", bufs=1))
        xb = sb.tile([P, n_ch * XW], BF16, tag="xb", name="xb")
        wq_sb = sb.tile([P, n_d * HDL], BF16, tag="wq", name="wq")
        wk_sb = sb.tile([P, n_d * HDL], BF16, tag="wk", name="wk")
        wv_sb = sb.tile([P, n_d * NV], BF16, tag="wv", name="wv")
        wp_sb = sb.tile([P, n_m * D], BF16, tag="wp", name="wp")
        qT = [sb.tile([P, S], BF16, tag=f"qT{m}", name=f"qT{m}") for m in range(n_m)]
        kT = [sb.tile([P, S], BF16, tag=f"kT{m}", name=f"kT{m}") for m in range(n_m)]
        Vt = [sb.tile([P, NV], BF16, tag=f"V{t}", name=f"V{t}") for t in range(n_t)]
        OT = [sb.tile([P, S], BF16, tag=f"OT{m}", name=f"OT{m}") for m in range(n_m)]
        bq_sb = sb.tile([P, n_m], F32, tag="bq", name="bq")
        bk_sb = sb.tile([P, n_m], F32, tag="bk", name="bk")
        bv_row = sb.tile([1, NV], F32, tag="bvr", name="bvr")
        bvb = sb.tile([P, NV], F32, tag="bvb", name="bvb")

        def xsl(c, d, off, w):
            return xb[:, c * XW + d * CH + off: c * XW + d * CH + off + w]

        pT_pool = ctx.enter_context(tc.tile_pool(name="pT", bufs=4))
        nrm = ctx.enter_context(tc.tile_pool(name="nrm", bufs=2))
        ystream = ctx.enter_context(tc.tile_pool(name="ystream", bufs=3))

        # Persistent PSUM: sc (2 banks x bufs=2) + pv (2 banks) = 6 banks.
        psA = ctx.enter_context(tc.tile_pool(name="psA", bufs=1, space="PSUM"))

        # HWLOOP=N wraps the whole body in a hardware loop for on-device
        # timing: per-iter time = (wall(N) - wall(1)) / (N - 1).
        HWLOOP = int(_os.environ.get("HWLOOP", "0"))
        if HWLOOP:
            ctx.enter_context(tc.For_i(0, HWLOOP))

        # ---- DMA queue: wk/chunk-0 halves first so PE starts ~2.5us in ----
        HK = n_d * HDL // 2
        nc.sync.dma_start(wk_sb[:, 0:HK], wk[:, 0:HK])
        nc.sync.dma_start(xb[:, 0:XW // 2], x[0:P, 0:XW // 2])
        nc.sync.dma_start(wk_sb[:, HK:2 * HK], wk[:, HK:2 * HK])
        nc.sync.dma_start(xb[:, XW // 2:XW], x[0:P, XW // 2:XW])
        nc.sync.dma_start(bq_sb[:], bq[:, :])
        nc.sync.dma_start(bk_sb[:], bk[:, :])
        nc.sync.dma_start(bv_row[:], bv[:, :])
        nc.gpsimd.partition_broadcast(bvb[:], bv_row[0:1, :])
        nc.sync.dma_start(wv_sb[:], wv[:, :])
        nc.sync.dma_start(wq_sb[:], wq[:, :])
        for c in range(1, n_ch):
            nc.sync.dma_start(xb[:, c * XW:(c + 1) * XW], x[c * P:(c + 1) * P, :])
        nc.sync.dma_start(wp_sb[:], wp[:, :])

        # sc and pT are [P, 2, QB]: the two heads' halves anchor at fixed
        # half*QB offsets so matmul outputs always start at a PSUM bank
        # boundary (mid-bank matmul outputs crash the runtime), and the exp
        # reads both halves in one (possibly strided) access pattern.
        def emit_sc(qlo, qw, m, tt):
            """Scores for one (query-range, head-pair, key-tile) -> PSUM tile."""
            sc = psA.tile([P, 2, QB], F32, tag="sc", name="sc", bufs=2)
            for half in (0, 1):
                plo = half * 64
                nc.tensor.matmul(
                    sc[:, half, 0:qw],
                    kT[m][plo:plo + 64, tt * P:(tt + 1) * P],
                    qT[m][plo:plo + 64, qlo:qlo + qw],
                    start=True, stop=True)
            return sc

        def emit_exp(sc, qw):
            pT = pT_pool.tile([P, 2, QB], BF16, tag="pT", name="pT", bufs=4)
            nc.scalar.activation(pT[:, :, 0:qw], sc[:, :, 0:qw],
                                 AF.Exp, scale=SCALE)
            return pT

        def emit_pv(qw, m, tt, pv, pT):
            for half in (0, 1):
                h = 2 * m + half
                nc.tensor.matmul(
                    pv[0:65, half * QB: half * QB + qw],
                    Vt[tt][:, h * 65:(h + 1) * 65],
                    pT[:, half, 0:qw],
                    start=(tt == 0), stop=(tt == n_t - 1))

        def emit_stripe_iter(qlo, qw, m, tt, pv):
            sc = emit_sc(qlo, qw, m, tt)
            pT = emit_exp(sc, qw)
            emit_pv(qw, m, tt, pv, pT)

        def emit_normalize(qlo, qw, m, pv):
            """OT[m][:, query-range] = pv[0:64] * bcast(1/pv[64]).

            pv is first copied to SBUF in one DVE op so the PSUM banks free
            ~1.2us after the last pv matmul instead of after the full
            recip->broadcast->mult chain (kills the stripe-boundary stall)."""
            # single copy frees the pv PSUM banks ~1.2us after the last pv
            # matmul; everything below reads the SBUF copy. For half-width
            # stripes copy only the written bank-anchored regions.
            pvS = nrm.tile([65, 2 * QB], F32, tag="pvS", name="pvS", bufs=2)
            if qw == QB:
                nc.vector.tensor_copy(pvS[:], pv[:])
            else:
                for half in (0, 1):
                    co = half * QB
                    nc.vector.tensor_copy(pvS[0:65, co:co + qw],
                                          pv[0:65, co:co + qw])
            dens = []
            for half in (0, 1):
                co = half * QB
                # denominator row to a partition-0 tile (gpsimd broadcast and
                # the custom-DVE reciprocal both need partition-0 sources)
                den = nrm.tile([1, QB], F32, tag=f"den{half}",
                               name=f"den{half}", bufs=2)
                nc.vector.tensor_copy(den[0:1, 0:qw], pvS[64:65, co:co + qw])
                dens.append(den)
            for half in (0, 1):
                plo = half * 64
                co = half * QB
                rb = nrm.tile([64, QB], F32, tag="rb", name="rb", bufs=2)
                nc.gpsimd.partition_broadcast(rb[0:64, 0:qw],
                                              dens[half][0:1, 0:qw])
                rc = nrm.tile([64, QB], F32, tag="rc", name="rc", bufs=2)
                nc.vector.reciprocal_approx_fast(rc[0:64, 0:qw], rb[0:64, 0:qw])
                nc.vector.tensor_tensor(
                    OT[m][plo:plo + 64, qlo:qlo + qw],
                    pvS[0:64, co:co + qw], rc[0:64, 0:qw], op=MUL)

        def emit_qkT(c, m, dst_list, w_big, b_sb, psum_pool, ptag, pbufs):
            """dst[m][:, chunk c] = W.T @ x + b (per-partition bias)."""
            clo = c * CH
            ps = psum_pool.tile([P, CH], F32, tag=ptag, name=ptag, bufs=pbufs)
            for dt in range(n_d):
                nc.tensor.matmul(ps[:], w_big[:, dt * HDL + m * P: dt * HDL + (m + 1) * P],
                                 xsl(c, dt, 0, CH),
                                 start=(dt == 0), stop=(dt == n_d - 1))
            nc.vector.tensor_scalar_add(out=dst_list[m][:, clo:clo + CH],
                                        in0=ps[:], scalar1=b_sb[:, m:m + 1])

        # ---- Phase A: kT/V~ chunk-by-chunk, qT chunk 0, stripe (qb0, m0) ----
        pv0 = psA.tile([65, 2 * QB], F32, tag="pv", name="pv", bufs=1)
        with tc.tile_pool(name="psB", bufs=1, space="PSUM") as psB:
            for c in range(n_ch):
                for m in range(n_m):
                    emit_qkT(c, m, kT, wk_sb, bk_sb, psB, "qkv", 2)
                for t in range(tpc):
                    tt = c * tpc + t
                    ps = psB.tile([P, NV], F32, tag="qkv", name="qkv", bufs=2)
                    for dt in range(n_d):
                        nc.tensor.matmul(ps[:], xsl(c, dt, t * P, P),
                                         wv_sb[:, dt * NV:(dt + 1) * NV],
                                         start=(dt == 0), stop=(dt == n_d - 1))
                    nc.vector.tensor_tensor(Vt[tt][:], ps[:], bvb[:], op=ADD)
                if c == 0:
                    for m in range(n_m):
                        emit_qkT(0, m, qT, wq_sb, bq_sb, psB, "qkv", 2)
                # interleave stripe (qb0, m0) for the key tiles just produced
                for t in range(tpc):
                    emit_stripe_iter(0, QB, 0, c * tpc + t, pv0)
        emit_normalize(0, QB, 0, pv0)

        # ---- Phase B: remaining stripes; qT chunks 1-3 and proj interleaved
        # as fine-grained PE fillers (one matmul per stripe iteration) so the
        # Activation engine never starves while a filler chain runs. ----
        with tc.tile_pool(name="psC", bufs=1, space="PSUM") as psC:
            def gen_qkT_filler(c, m):
                """Yield per-matmul steps of qT[m][:, chunk c] production."""
                clo = c * CH
                ps = psC.tile([P, CH], F32, tag="y", name="y", bufs=2)
                for dt in range(n_d):
                    nc.tensor.matmul(
                        ps[:], wq_sb[:, dt * HDL + m * P: dt * HDL + (m + 1) * P],
                        xsl(c, dt, 0, CH),
                        start=(dt == 0), stop=(dt == n_d - 1))
                    if dt < n_d - 1:
                        yield
                nc.vector.tensor_scalar_add(out=qT[m][:, clo:clo + CH],
                                            in0=ps[:], scalar1=bq_sb[:, m:m + 1])
                yield

            def gen_proj_filler(qlo, qw):
                """Yield per-matmul steps of the [qlo, qlo+qw) projection."""
                for qt in range(qw // P):
                    rlo = qlo + qt * P
                    yt = ystream.tile([P, D], F32, tag="yt", name="yt", bufs=3)
                    for nn in (0, 512):
                        yp = psC.tile([P, 512], F32, tag="y", name="y", bufs=2)
                        for m in range(n_m):
                            nc.tensor.matmul(yp[:], OT[m][:, rlo:rlo + P],
                                             wp_sb[:, m * D + nn: m * D + nn + 512],
                                             start=(m == 0), stop=(m == n_m - 1))
                            if m < n_m - 1:
                                yield
                        nc.vector.tensor_copy(yt[:, nn:nn + 512], yp[:])
                        yield
                    nc.sync.dma_start(y[rlo:rlo + P, :], yt[:])

            def chain(*gens):
                for g in gens:
                    yield from g

            def drain(filler):
                if filler is not None:
                    for _ in filler:
                        pass

            # (stripe, filler, start_at) schedule: qT chunk c lands during the
            # stripe before q-block c; proj(qb) lands during the stripe after
            # (qb, m1), delayed 6 iters so its first matmul never waits on the
            # normalize chain. The last q-block's m1 stripe is split into two
            # 256-query half-stripes so the final normalize+projection overlap
            # the second half's compute and the tail shrinks to ~5us.
            sched = [
                ((0, QB, 1), chain(gen_qkT_filler(1, 0), gen_qkT_filler(1, 1)), 0),
                ((QB, QB, 0), gen_proj_filler(0, QB), 6),
                ((QB, QB, 1), chain(gen_qkT_filler(2, 0), gen_qkT_filler(2, 1)), 0),
                ((2 * QB, QB, 0), gen_proj_filler(QB, QB), 6),
                ((2 * QB, QB, 1), chain(gen_qkT_filler(3, 0), gen_qkT_filler(3, 1)), 0),
                ((3 * QB, QB, 0), gen_proj_filler(2 * QB, QB), 6),
                ((3 * QB, QB // 2, 1), None, 0),
                ((3 * QB + QB // 2, QB // 2, 1),
                 gen_proj_filler(3 * QB, QB // 2), 2),
            ]
            # Software-pipelined flat loop: the NEXT iteration's scores are
            # emitted on PE before this iteration's pv, so the PE computes
            # sc(i+1) during exp(i) and the exp->pv->sc->exp latency chain
            # never gates the Activation engine.
            flat = []
            for (qlo, qw, m), filler, start_at in sched:
                for tt in range(n_t):
                    flat.append((qlo, qw, m, tt, filler, start_at))
            pv_tiles = {}
            sc_cur = emit_sc(*flat[0][:4]) if flat else None
            for gi, (qlo, qw, m, tt, filler, start_at) in enumerate(flat):
                pT = emit_exp(sc_cur, qw)
                if gi + 1 < len(flat):
                    sc_cur = emit_sc(*flat[gi + 1][:4])
                key = (qlo, m)
                if key not in pv_tiles:
                    pv_tiles[key] = psA.tile([65, 2 * QB], F32, tag="pv",
                                             name="pv", bufs=1)
                emit_pv(qw, m, tt, pv_tiles[key], pT)
                if filler is not None and tt >= start_at:
                    n_fill = 16 - start_at
                    steps = (16 * (tt - start_at + 1) + n_fill - 1) // n_fill \
                        - (16 * (tt - start_at) + n_fill - 1) // n_fill
                    for _ in range(max(1, steps)):
                        next(filler, None)
                if tt == n_t - 1:
                    emit_normalize(qlo, qw, m, pv_tiles[key])
                    drain(filler)
            drain(gen_proj_filler(3 * QB + QB // 2, QB // 2))

    nc.compile()
    return nc


def _get_nc():
    if "nc" not in _cache:
        _cache["nc"] = _build()
    return _cache["nc"]


def make_in_maps(inputs, Wkv, bkv, Wq, bq, Wp, bp):
    """Host-side sharding: per-core input dicts (bf16, chunk-major layouts)."""
    BF = ml_dtypes.bfloat16
    n_d, n_m, n_ch = D // P, HDL // P, S // CH
    inputs = np.asarray(inputs, dtype=np.float32)
    Wkv = np.asarray(Wkv, dtype=np.float32)
    bkv = np.asarray(bkv, dtype=np.float32)
    Wq = np.asarray(Wq, dtype=np.float32)
    bq = np.asarray(bq, dtype=np.float32)
    Wp = np.asarray(Wp, dtype=np.float32)

    def dmaj(w, width):
        # [D, width] -> [128, n_d*width] rows: row p = [d0 | d1 | ...]
        return np.ascontiguousarray(
            w.reshape(n_d, P, width).transpose(1, 0, 2).reshape(P, n_d * width))

    in_maps = []
    for c in range(N_CORES):
        b = c // CORES_PER_B
        g = c % CORES_PER_B
        hsl = slice(g * HDL, (g + 1) * HDL)
        X = inputs[:, b, :].T                      # [D, S]
        # [d, p, ch, j] -> [ch, p, d, j] -> [n_ch*128, n_d*CH]
        x_c = np.ascontiguousarray(
            X.reshape(n_d, P, n_ch, CH).transpose(2, 1, 0, 3)
            .reshape(n_ch * P, n_d * CH)).astype(BF)
        wq_c = dmaj(Wq[:, hsl], HDL).astype(BF)
        wk_c = dmaj(Wkv[:, hsl], HDL).astype(BF)
        bq_c = np.ascontiguousarray(bq[hsl].reshape(n_m, P).T)
        bk_c = np.ascontiguousarray(bkv[hsl].reshape(n_m, P).T)
        wv_full = Wkv[:, H * HD + g * HDL: H * HD + (g + 1) * HDL]
        bv_full = bkv[H * HD + g * HDL: H * HD + (g + 1) * HDL]
        wv_c = np.zeros((D, NV), dtype=np.float32)
        bv_c = np.zeros((1, NV), dtype=np.float32)
        for h in range(NH):
            wv_c[:, h * 65:h * 65 + 64] = wv_full[:, h * 64:(h + 1) * 64]
            bv_c[0, h * 65:h * 65 + 64] = bv_full[h * 64:(h + 1) * 64]
            bv_c[0, h * 65 + 64] = 1.0
        wv_c = dmaj(wv_c, NV).astype(BF)
        # Wp [HDL, D] -> [128, n_m*D]
        wp_c = np.ascontiguousarray(
            Wp[hsl, :].reshape(n_m, P, D).transpose(1, 0, 2).reshape(P, n_m * D)
        ).astype(BF)
        in_maps.append(dict(
            x=x_c, wq=wq_c, wk=wk_c, wv=wv_c,
            bq=bq_c, bk=bk_c, bv=bv_c, wp=wp_c))
    return in_maps


def combine_outputs(results, bp):
    """Host-side unshard: sum head-group partials per batch, add bp."""
    bp = np.asarray(bp, dtype=np.float32)
    out = np.zeros((S, B, D), dtype=np.float32)
    for b in range(B):
        acc = results[b * CORES_PER_B]["y"].copy()
        for g in range(1, CORES_PER_B):
            acc += results[b * CORES_PER_B + g]["y"]
        out[:, b, :] = acc + bp
    return out


def kernel(inputs, Wkv, bkv, Wq, bq, Wp, bp):
    from concourse.bass_utils import run_bass_kernel_spmd
    nc = _get_nc()
    in_maps = make_in_maps(inputs, Wkv, bkv, Wq, bq, Wp, bp)
    res = run_bass_kernel_spmd(nc, in_maps, list(range(N_CORES)))
    return combine_outputs(res.results, bp)
